# revision 16
# baseline (speedup 1.0000x reference)
"""CenterLoss kernel for Trainium2 (Bass/Tile), 8-core data-parallel.

loss = sum_i ||x_i - centers[labels_i]||^2
  x: (65536, 512) f32, labels: (65536,) int, centers: (512, 512) f32

Per-core plan (8192 rows each), using the expansion
  loss = sum x^2 - 2*sum_{c,d} S[c,d]*centers[c,d] + sum_c count_c*||C_c||^2
with S = onehot(labels)^T @ x and count_c the label histogram, both computed
on the PE via one-hot matmuls (exactly representable in bf16):
  - x streamed HBM->SBUF with an in-flight f32->fp8 cast (SWDGE); the stream
    runs at the per-core HBM roofline (~358 GB/s), so the schedule minimizes
    head (small first DMA) and tail (small last segments, per-chunk PSUM
    contraction, PE partition-reduce into a single-partition output).
  - DVE builds the one-hot tile: is_equal(iota_row, label_p)
  - PE: per 128-row tile pair, 4 matmuls accumulate S chunks into 4 separate
    PSUM banks and 4 N=1 matmuls accumulate the histogram
  - ACT accumulates sum(x^2) per segment into a shared partials tile P
  - tail: per-chunk S*C contraction (starts as soon as that chunk's last
    matmul retires), count*csq, then ones^T @ P on the PE collapses all
    partials across partitions to one [1,NPART] row -> single small DMA out.
"""

import sys

import numpy as np

sys.path.insert(0, "/opt/trn_rl_repo")

N_CORES = 8
B = 65536
D = 512
B_L = B // N_CORES  # 8192 rows per core
# DMA/compute segments (rows): small head to start the DMA queue early,
# small tail segments so little PE/ACT work remains after the last byte.
SEGMENTS = [256] + [512] * 14 + [256] * 3
assert sum(SEGMENTS) == B_L
N_SEG = len(SEGMENTS)
N_TILES = B_L // 128  # 64
NCH = D // 128  # 4 class chunks
# partial columns: x^2 per segment, -2*S.C per chunk, cnt*csq per chunk
NPART = N_SEG + 2 * NCH

_CACHE = {}


def _build():
    """Trace the Bass/Tile program once; returns the compiled Bacc module."""
    if "nc" in _CACHE:
        return _CACHE["nc"]

    import concourse.bacc as bacc
    import concourse.mybir as mybir
    import concourse.tile as tile

    f32 = mybir.dt.float32
    bf16 = mybir.dt.bfloat16
    fp8 = mybir.dt.float8e4

    nc = bacc.Bacc("TRN2", debug=False, num_devices=N_CORES)
    x_t = nc.dram_tensor("x", [B_L, D], f32, kind="ExternalInput")
    iota_t = nc.dram_tensor("iota16", [128, D], mybir.dt.float16, kind="ExternalInput")
    labf_t = nc.dram_tensor("labf", [128, N_TILES], f32, kind="ExternalInput")
    # centers uploaded pre-cast to bf16: halves the HWDGE backlog that delays
    # the x-stream start (cross-term precision loss ~0.1%, tolerance is 2e-2)
    c_t = nc.dram_tensor("centers16", [D, D], bf16, kind="ExternalInput")
    out_t = nc.dram_tensor("out", [1, NPART], f32, kind="ExternalOutput")

    with tile.TileContext(nc) as tc:
        with (
            tc.tile_pool(name="io", bufs=12) as io_pool,
            tc.tile_pool(name="oh", bufs=8) as oh_pool,
            tc.tile_pool(name="psum", bufs=1, space="PSUM") as psum_pool,
            tc.tile_pool(name="misc", bufs=1) as misc_pool,
        ):
            # centers first on the HWDGE queue: its 1MB must be fully drained
            # before the SWDGE x-stream ramps (~9us) or the two queues
            # round-robin the same 16 SDMA engines and the stream slows down
            # "(p n)" keeps each partition's source rows contiguous in HBM;
            # the iota permutation below makes the one-hot chunks match.
            cent_sb = misc_pool.tile([128, NCH, D], bf16)
            nc.sync.dma_start(
                cent_sb[:], c_t.ap().rearrange("(p n) d -> p n d", n=NCH)
            )
            iota_sb = misc_pool.tile([128, D], mybir.dt.float16)
            nc.sync.dma_start(iota_sb[:], iota_t.ap())
            labf_sb = misc_pool.tile([128, N_TILES], f32)
            nc.sync.dma_start(labf_sb[:], labf_t.ap())
            ones8 = misc_pool.tile([128, 2, 1], fp8)
            nc.vector.memset(ones8[:], 1.0)
            onesf = misc_pool.tile([128, 1], f32)
            nc.vector.memset(onesf[:], 1.0)

            # all scalar partials live in one [128, NPART] tile; the final
            # PE reduce collapses it across partitions in one matmul
            P = misc_pool.tile([128, NPART], f32)
            csq_col = misc_pool.tile([128, NCH], f32)
            junk_dve = misc_pool.tile([128, 1], f32)
            junk_act = misc_pool.tile([128, 1], f32)
            out_sb = misc_pool.tile([1, NPART], f32)

            # one PSUM bank per class chunk so each chunk's S*C contraction
            # can start the moment that chunk's accumulation stops
            S_ps = [
                psum_pool.tile([128, D], f32, name=f"S_ps{c}") for c in range(NCH)
            ]
            count_ps = [
                psum_pool.tile([128, 1], f32, tag=f"cnt{c}", name=f"count_ps{c}")
                for c in range(NCH)
            ]

            # csq: scheduled mid-stream (tail-only consumers)
            for c in range(NCH):
                nc.scalar.activation(
                    junk_act[:].broadcast_to(cent_sb[:, c, :].shape),
                    cent_sb[:, c, :],
                    mybir.ActivationFunctionType.Square,
                    accum_out=csq_col[:, c : c + 1],
                )

            x_ap = x_t.ap()
            row = 0
            tile_idx = 0
            for si, rows in enumerate(SEGMENTS):
                q = rows // 128
                x_sb = io_pool.tile([128, q, D], fp8, tag="x")
                # SWDGE casts f32 -> fp8e4m3 in flight; the first segment is
                # split so the DMA queue starts draining as early as possible
                if si == 0:
                    for h in range(q):
                        nc.gpsimd.dma_start(
                            x_sb[:, h : h + 1, :],
                            x_ap[row + h * 128 : row + (h + 1) * 128, :].rearrange(
                                "(q p) d -> p q d", p=128
                            ),
                        )
                else:
                    nc.gpsimd.dma_start(
                        x_sb[:],
                        x_ap[row : row + rows, :].rearrange(
                            "(q p) d -> p q d", p=128
                        ),
                    )
                for j in range(q // 2):
                    oh = oh_pool.tile([128, 2, D], fp8, tag="oh")
                    for u in range(2):
                        t = tile_idx + 2 * j + u
                        nc.vector.tensor_scalar(
                            out=oh[:, u, :],
                            in0=iota_sb[:],
                            scalar1=labf_sb[:, t : t + 1],
                            scalar2=None,
                            op0=mybir.AluOpType.is_equal,
                        )
                    first = si == 0 and j == 0
                    last = si == N_SEG - 1 and j == q // 2 - 1
                    if last:
                        # emit all S matmuls before the count matmuls so each
                        # chunk's S*C contraction unblocks as early as possible
                        for c in range(NCH):
                            nc.tensor.matmul(
                                S_ps[c][:],
                                lhsT=oh[:, :, c * 128 : (c + 1) * 128],
                                rhs=x_sb[:, 2 * j : 2 * j + 2, :],
                                start=first,
                                stop=last,
                                perf_mode=mybir.MatmulPerfMode.DoubleRow,
                            )
                        for c in range(NCH):
                            nc.tensor.matmul(
                                count_ps[c][:],
                                lhsT=oh[:, :, c * 128 : (c + 1) * 128],
                                rhs=ones8[:],
                                start=first,
                                stop=last,
                                perf_mode=mybir.MatmulPerfMode.DoubleRow,
                            )
                    else:
                        for c in range(NCH):
                            nc.tensor.matmul(
                                S_ps[c][:],
                                lhsT=oh[:, :, c * 128 : (c + 1) * 128],
                                rhs=x_sb[:, 2 * j : 2 * j + 2, :],
                                start=first,
                                stop=last,
                                perf_mode=mybir.MatmulPerfMode.DoubleRow,
                            )
                            nc.tensor.matmul(
                                count_ps[c][:],
                                lhsT=oh[:, :, c * 128 : (c + 1) * 128],
                                rhs=ones8[:],
                                start=first,
                                stop=last,
                                perf_mode=mybir.MatmulPerfMode.DoubleRow,
                            )
                # sum(x^2) on ACT, one op per segment
                x_flat = x_sb[:].rearrange("p q d -> p (q d)")
                nc.scalar.activation(
                    junk_act[:].broadcast_to(x_flat.shape),
                    x_flat,
                    mybir.ActivationFunctionType.Square,
                    accum_out=P[:, si : si + 1],
                )
                row += rows
                tile_idx += q

            # tail: per-chunk r2_c = -2*sum_d S_c[p,d]*C_c[p,d] fused DVE ops;
            # chunk c's op only waits on chunk c's last matmul
            for c in range(NCH):
                nc.vector.scalar_tensor_tensor(
                    out=junk_dve[:].broadcast_to(S_ps[c][:].shape),
                    in0=S_ps[c][:],
                    scalar=-2.0,
                    in1=cent_sb[:, c, :],
                    op0=mybir.AluOpType.mult,
                    op1=mybir.AluOpType.mult,
                    accum_out=P[:, N_SEG + c : N_SEG + c + 1],
                )
            # r3_c = count_c * csq_c straight from PSUM (no staging copies)
            for c in range(NCH):
                nc.vector.scalar_tensor_tensor(
                    out=junk_dve[:],
                    in0=count_ps[c][:],
                    scalar=1.0,
                    in1=csq_col[:, c : c + 1],
                    op0=mybir.AluOpType.bypass,
                    op1=mybir.AluOpType.mult,
                    accum_out=P[:, N_SEG + NCH + c : N_SEG + NCH + c + 1],
                )
            # collapse partitions on the PE: ones^T @ P -> [1, NPART] on one
            # partition, then one tiny contiguous DMA out (single descriptor,
            # single completion receipt instead of 128 scattered 4B writes)
            psum_red = psum_pool.tile([1, NPART], f32, tag="cnt0", name="psum_red")
            nc.tensor.matmul(
                psum_red[:], lhsT=onesf[:], rhs=P[:], start=True, stop=True
            )
            nc.vector.tensor_copy(out_sb[:], psum_red[:])
            nc.sync.dma_start(out_t.ap(), out_sb[:])

    nc.compile()
    _CACHE["nc"] = nc
    return nc


def _prep_inputs(x, labels, centers):
    """Shard full inputs into the 8 per-core input maps."""
    x = np.asarray(x, dtype=np.float32)
    labels = np.asarray(labels)
    centers = np.ascontiguousarray(np.asarray(centers, dtype=np.float32))
    # column n*128+j holds class 4j+n, matching the "(p n)" centers layout
    # (partition p of chunk n <-> class 4p+n)
    col = np.arange(D)
    perm = (4 * (col % 128) + col // 128).astype(np.float16)
    iota16 = np.ascontiguousarray(np.tile(perm, (128, 1)))
    import ml_dtypes

    cent16 = np.ascontiguousarray(centers.astype(ml_dtypes.bfloat16))
    in_maps = []
    for c in range(N_CORES):
        xs = np.ascontiguousarray(x[c * B_L : (c + 1) * B_L])
        lab = labels[c * B_L : (c + 1) * B_L]
        # labf[p, t] = label of row t*128+p, as exact small-int f32
        labf = np.ascontiguousarray(
            lab.reshape(N_TILES, 128).T.astype(np.float32)
        )
        in_maps.append(
            {"x": xs, "iota16": iota16, "labf": labf, "centers16": cent16}
        )
    return in_maps


def _run(x, labels, centers, trace=False):
    from concourse import bass_utils

    nc = _build()
    in_maps = _prep_inputs(x, labels, centers)
    res = bass_utils.run_bass_kernel_spmd(
        nc, in_maps, core_ids=list(range(N_CORES)), trace=trace
    )
    total = np.float64(0.0)
    for r in res.results:
        total += np.sum(r["out"].astype(np.float64))
    return np.array(total, dtype=np.float32), res


def kernel(x, labels, centers):
    out, _ = _run(x, labels, centers, trace=False)
    return out


def kernel_traced(x, labels, centers):
    return _run(x, labels, centers, trace=True)


# revision 17
# speedup vs baseline: 1.0046x; 1.0046x over previous
"""CenterLoss kernel for Trainium2 (Bass/Tile), 8-core data-parallel.

loss = sum_i ||x_i - centers[labels_i]||^2
  x: (65536, 512) f32, labels: (65536,) int, centers: (512, 512) f32

Per-core plan (8192 rows each), using the expansion
  loss = sum x^2 - 2*sum_{c,d} S[c,d]*centers[c,d] + sum_c count_c*||C_c||^2
with S = onehot(labels)^T @ x and count_c the label histogram, both computed
on the PE via one-hot matmuls (exactly representable in bf16):
  - x streamed HBM->SBUF with an in-flight f32->fp8 cast (SWDGE); the stream
    runs at the per-core HBM roofline (~358 GB/s), so the schedule minimizes
    head (small first DMA) and tail (small last segments, per-chunk PSUM
    contraction, PE partition-reduce into a single-partition output).
  - DVE builds the one-hot tile: is_equal(iota_row, label_p)
  - PE: per 128-row tile pair, 4 matmuls accumulate S chunks into 4 separate
    PSUM banks and 4 N=1 matmuls accumulate the histogram
  - ACT accumulates sum(x^2) per segment into a shared partials tile P
  - tail: per-chunk S*C contraction (starts as soon as that chunk's last
    matmul retires), count*csq, then ones^T @ P on the PE collapses all
    partials across partitions to one [1,NPART] row -> single small DMA out.
"""

import sys

import numpy as np

sys.path.insert(0, "/opt/trn_rl_repo")

N_CORES = 8
B = 65536
D = 512
B_L = B // N_CORES  # 8192 rows per core
# DMA/compute segments (rows): small head to start the DMA queue early,
# small tail segments so little PE/ACT work remains after the last byte.
SEGMENTS = [256] + [512] * 14 + [256] * 3
assert sum(SEGMENTS) == B_L
N_SEG = len(SEGMENTS)
N_TILES = B_L // 128  # 64
NCH = D // 128  # 4 class chunks
# partial columns: x^2 per segment, -2*S.C per chunk, cnt*csq per chunk
NPART = N_SEG + 2 * NCH

_CACHE = {}


def _build():
    """Trace the Bass/Tile program once; returns the compiled Bacc module."""
    if "nc" in _CACHE:
        return _CACHE["nc"]

    import concourse.bacc as bacc
    import concourse.mybir as mybir
    import concourse.tile as tile

    f32 = mybir.dt.float32
    bf16 = mybir.dt.bfloat16
    fp8 = mybir.dt.float8e4

    nc = bacc.Bacc("TRN2", debug=False, num_devices=N_CORES)
    x_t = nc.dram_tensor("x", [B_L, D], f32, kind="ExternalInput")
    iota_t = nc.dram_tensor("iota16", [128, D], mybir.dt.float16, kind="ExternalInput")
    labf_t = nc.dram_tensor("labf", [128, N_TILES], f32, kind="ExternalInput")
    # centers uploaded pre-cast to bf16: halves the HWDGE backlog that delays
    # the x-stream start (cross-term precision loss ~0.1%, tolerance is 2e-2)
    c_t = nc.dram_tensor("centers16", [D, D], bf16, kind="ExternalInput")
    out_t = nc.dram_tensor("out", [1, NPART], f32, kind="ExternalOutput")

    with tile.TileContext(nc) as tc:
        with (
            tc.tile_pool(name="io", bufs=12) as io_pool,
            tc.tile_pool(name="oh", bufs=8) as oh_pool,
            tc.tile_pool(name="psum", bufs=1, space="PSUM") as psum_pool,
            tc.tile_pool(name="misc", bufs=1) as misc_pool,
        ):
            # centers first on the HWDGE queue: its 1MB must be fully drained
            # before the SWDGE x-stream ramps (~9us) or the two queues
            # round-robin the same 16 SDMA engines and the stream slows down
            # "(p n)" keeps each partition's source rows contiguous in HBM;
            # the iota permutation below makes the one-hot chunks match.
            cent_sb = misc_pool.tile([128, NCH, D], bf16)
            nc.sync.dma_start(
                cent_sb[:], c_t.ap().rearrange("(p n) d -> p n d", n=NCH)
            )
            iota_sb = misc_pool.tile([128, D], mybir.dt.float16)
            nc.sync.dma_start(iota_sb[:], iota_t.ap())
            labf_sb = misc_pool.tile([128, N_TILES], f32)
            nc.sync.dma_start(labf_sb[:], labf_t.ap())
            ones8 = misc_pool.tile([128, 2, 1], fp8)
            nc.vector.memset(ones8[:], 1.0)
            onesf = misc_pool.tile([128, 1], f32)
            nc.vector.memset(onesf[:], 1.0)

            # all scalar partials live in one [128, NPART] tile; the final
            # PE reduce collapses it across partitions in one matmul
            P = misc_pool.tile([128, NPART], f32)
            csq_col = misc_pool.tile([128, NCH], f32)
            junk_dve = misc_pool.tile([128, 1], f32)
            junk_act = misc_pool.tile([128, 1], f32)
            out_sb = misc_pool.tile([1, NPART], f32)

            # one PSUM bank per class chunk so each chunk's S*C contraction
            # can start the moment that chunk's accumulation stops
            S_ps = [
                psum_pool.tile([128, D], f32, name=f"S_ps{c}") for c in range(NCH)
            ]
            count_ps = [
                psum_pool.tile([128, 1], f32, tag=f"cnt{c}", name=f"count_ps{c}")
                for c in range(NCH)
            ]

            # csq: scheduled mid-stream (tail-only consumers)
            for c in range(NCH):
                nc.scalar.activation(
                    junk_act[:].broadcast_to(cent_sb[:, c, :].shape),
                    cent_sb[:, c, :],
                    mybir.ActivationFunctionType.Square,
                    accum_out=csq_col[:, c : c + 1],
                )

            x_ap = x_t.ap()
            row = 0
            tile_idx = 0
            for si, rows in enumerate(SEGMENTS):
                q = rows // 128
                x_sb = io_pool.tile([128, q, D], fp8, tag="x")
                # SWDGE casts f32 -> fp8e4m3 in flight; the first segment is
                # split so the DMA queue starts draining as early as possible
                if si == 0:
                    for h in range(q):
                        nc.gpsimd.dma_start(
                            x_sb[:, h : h + 1, :],
                            x_ap[row + h * 128 : row + (h + 1) * 128, :].rearrange(
                                "(q p) d -> p q d", p=128
                            ),
                        )
                else:
                    nc.gpsimd.dma_start(
                        x_sb[:],
                        x_ap[row : row + rows, :].rearrange(
                            "(q p) d -> p q d", p=128
                        ),
                    )
                for j in range(q // 2):
                    oh = oh_pool.tile([128, 2, D], fp8, tag="oh")
                    for u in range(2):
                        t = tile_idx + 2 * j + u
                        nc.vector.tensor_scalar(
                            out=oh[:, u, :],
                            in0=iota_sb[:],
                            scalar1=labf_sb[:, t : t + 1],
                            scalar2=None,
                            op0=mybir.AluOpType.is_equal,
                        )
                    first = si == 0 and j == 0
                    last = si == N_SEG - 1 and j == q // 2 - 1
                    if last:
                        # emit all S matmuls before the count matmuls so each
                        # chunk's S*C contraction unblocks as early as possible
                        for c in range(NCH):
                            nc.tensor.matmul(
                                S_ps[c][:],
                                lhsT=oh[:, :, c * 128 : (c + 1) * 128],
                                rhs=x_sb[:, 2 * j : 2 * j + 2, :],
                                start=first,
                                stop=last,
                                perf_mode=mybir.MatmulPerfMode.DoubleRow,
                            )
                        for c in range(NCH):
                            nc.tensor.matmul(
                                count_ps[c][:],
                                lhsT=oh[:, :, c * 128 : (c + 1) * 128],
                                rhs=ones8[:],
                                start=first,
                                stop=last,
                                perf_mode=mybir.MatmulPerfMode.DoubleRow,
                            )
                    else:
                        for c in range(NCH):
                            nc.tensor.matmul(
                                S_ps[c][:],
                                lhsT=oh[:, :, c * 128 : (c + 1) * 128],
                                rhs=x_sb[:, 2 * j : 2 * j + 2, :],
                                start=first,
                                stop=last,
                                perf_mode=mybir.MatmulPerfMode.DoubleRow,
                            )
                            nc.tensor.matmul(
                                count_ps[c][:],
                                lhsT=oh[:, :, c * 128 : (c + 1) * 128],
                                rhs=ones8[:],
                                start=first,
                                stop=last,
                                perf_mode=mybir.MatmulPerfMode.DoubleRow,
                            )
                # sum(x^2) on ACT, one op per segment
                x_flat = x_sb[:].rearrange("p q d -> p (q d)")
                nc.scalar.activation(
                    junk_act[:].broadcast_to(x_flat.shape),
                    x_flat,
                    mybir.ActivationFunctionType.Square,
                    accum_out=P[:, si : si + 1],
                )
                row += rows
                tile_idx += q

            # tail: per-chunk r2_c = -2*sum_d S_c[p,d]*C_c[p,d] fused DVE ops;
            # chunk c's op only waits on chunk c's last matmul
            for c in range(NCH):
                nc.vector.scalar_tensor_tensor(
                    out=junk_dve[:].broadcast_to(S_ps[c][:].shape),
                    in0=S_ps[c][:],
                    scalar=-2.0,
                    in1=cent_sb[:, c, :],
                    op0=mybir.AluOpType.mult,
                    op1=mybir.AluOpType.mult,
                    accum_out=P[:, N_SEG + c : N_SEG + c + 1],
                )
            # r3 = sum_c count_c * csq_c (per partition-class)
            cnt_col = misc_pool.tile([128, NCH], f32)
            for c in range(NCH):
                nc.vector.tensor_copy(cnt_col[:, c : c + 1], count_ps[c][:])
            nc.vector.scalar_tensor_tensor(
                out=junk_dve[:].broadcast_to(cnt_col[:].shape),
                in0=cnt_col[:],
                scalar=1.0,
                in1=csq_col[:],
                op0=mybir.AluOpType.bypass,
                op1=mybir.AluOpType.mult,
                accum_out=P[:, N_SEG + NCH : N_SEG + NCH + 1],
            )
            # collapse partitions on the PE: ones^T @ P -> [1, NPART] on one
            # partition, then one tiny contiguous DMA out (single descriptor,
            # single completion receipt instead of 128 scattered 4B writes)
            psum_red = psum_pool.tile([1, NPART], f32, tag="cnt0", name="psum_red")
            nc.tensor.matmul(
                psum_red[:], lhsT=onesf[:], rhs=P[:], start=True, stop=True
            )
            nc.vector.tensor_copy(out_sb[:], psum_red[:])
            nc.sync.dma_start(out_t.ap(), out_sb[:])

    nc.compile()
    _CACHE["nc"] = nc
    return nc


def _prep_inputs(x, labels, centers):
    """Shard full inputs into the 8 per-core input maps."""
    x = np.asarray(x, dtype=np.float32)
    labels = np.asarray(labels)
    centers = np.ascontiguousarray(np.asarray(centers, dtype=np.float32))
    # column n*128+j holds class 4j+n, matching the "(p n)" centers layout
    # (partition p of chunk n <-> class 4p+n)
    col = np.arange(D)
    perm = (4 * (col % 128) + col // 128).astype(np.float16)
    iota16 = np.ascontiguousarray(np.tile(perm, (128, 1)))
    import ml_dtypes

    cent16 = np.ascontiguousarray(centers.astype(ml_dtypes.bfloat16))
    in_maps = []
    for c in range(N_CORES):
        xs = np.ascontiguousarray(x[c * B_L : (c + 1) * B_L])
        lab = labels[c * B_L : (c + 1) * B_L]
        # labf[p, t] = label of row t*128+p, as exact small-int f32
        labf = np.ascontiguousarray(
            lab.reshape(N_TILES, 128).T.astype(np.float32)
        )
        in_maps.append(
            {"x": xs, "iota16": iota16, "labf": labf, "centers16": cent16}
        )
    return in_maps


def _run(x, labels, centers, trace=False):
    from concourse import bass_utils

    nc = _build()
    in_maps = _prep_inputs(x, labels, centers)
    res = bass_utils.run_bass_kernel_spmd(
        nc, in_maps, core_ids=list(range(N_CORES)), trace=trace
    )
    total = np.float64(0.0)
    for r in res.results:
        total += np.sum(r["out"].astype(np.float64))
    return np.array(total, dtype=np.float32), res


def kernel(x, labels, centers):
    out, _ = _run(x, labels, centers, trace=False)
    return out


def kernel_traced(x, labels, centers):
    return _run(x, labels, centers, trace=True)
